# revision 1
# baseline (speedup 1.0000x reference)
"""Bezier Gaussian-splat raster kernel for 8 Trainium2 NeuronCores.

Problem: control_points [16,4,4,2] f32, sigma scalar f32 ->
raster [16,4,1,512,512] f32 where
  raster[b,s,0,p,q] = sum_t exp(-((y_t-g_p)^2+(x_t-g_q)^2)/(2 sigma^2))
with (x_t,y_t) the cubic Bezier curve sampled at 128 points and
g = arange(512)/512.

Strategy (data-parallel, no cross-core comms):
  - 16 batches / 8 cores -> 2 batches = 8 strokes per core.
  - Ax[t,q] = exp(-(x_t-g_q)^2 * inv) via Derivative_Erf(u) =
    2/sqrt(pi)*exp(-u^2); u built either inside the activation (per-stroke
    bias/scale APs) or by DVE tensor_scalar into f16 tiles feeding batched
    multi-stroke activations (fewer ACT instructions).
  - raster chunk = Ay[:,128p-chunk].T @ Ax on the tensor engine (fp16 in,
    fp32 PSUM out), 4 chunks per stroke into one 4-bank PSUM tile.
  - PSUM->SBUF drains scaled by pi/4, cast to fp16, split between the
    scalar and vector engines (knob-tuned for engine balance).
  - fp16 output halves HBM write traffic (the f32 roofline): one 512KiB
    HWDGE DMA per stroke; the host reassembles rows and upcasts to f32
    (rel tolerance 2e-2 vs fp16's ~3e-4 quantization error).
"""

import math

import numpy as np

import concourse.bass as bass
import concourse.mybir as mybir
import concourse.tile as tile
from concourse import bacc
from concourse.bass_utils import run_bass_kernel_spmd

RES = 512
STEPS = 128
NK = 4            # control points per stroke
B_FULL = 16
S_FULL = 4
N_CORES = 8
BPC = B_FULL // N_CORES      # batches per core
SPC = BPC * S_FULL           # strokes per core
PCHUNKS = RES // 128         # 128-row chunks of the raster

F16 = mybir.dt.float16
F32 = mybir.dt.float32
AF = mybir.ActivationFunctionType
ALU = mybir.AluOpType

PI_OVER_4 = math.pi / 4.0
SQRT2 = math.sqrt(2.0)

# Tuned via TimelineSim sweep (sim.py / sweep.py); see build_bass for
# knob meanings.
KNOBS = dict(
    exp_mode="perstroke",   # 'perstroke' | 'bigexpN' (u on DVE, N strokes/act)
    drain_split=576,        # elems of each stroke drain on ACT (rest DVE);
                            # covers chunk 0 (ready after its matmul) + a
                            # sliver of chunk 1 — ACT has slack once the
                            # grid is pre-scaled
    act_drain_strokes=(),   # whole-stroke ACT drains (used if drain_split=0)
    dma_group=2,            # strokes per output DMA (1MB transfers; ties
                            # group=8 on throughput, pipelines the DMA tail)
    apool_bufs=10,
    opool_bufs=4,
    upool_bufs=3,
    prescale_grid=True,     # grid pre-multiplied by sinv/RES at setup; each
                            # activation then uses an immediate scale of 1.0
                            # (one fewer AP operand per activation)
)


def _bernstein() -> np.ndarray:
    t = np.linspace(0.0, 1.0, STEPS, dtype=np.float64)
    rows = [math.comb(NK - 1, k) * t ** (NK - 1 - k) * (1.0 - t) ** k
            for k in range(NK)]
    return np.stack(rows).astype(np.float32)  # [4, 128] = feat[k, t]


def build_bass(repeats: int = 1, probe: str = "", **over) -> bass.Bass:
    """Build the per-core Bass program. `repeats` re-runs the whole stroke
    loop N times (same outputs) — used only by the timing harness to
    estimate steady-state per-iteration HW time from wall-clock deltas."""
    kn = dict(KNOBS, **over)
    exp_mode = kn["exp_mode"]
    big_n = int(exp_mode[6:]) if exp_mode.startswith("bigexp") else 0
    dma_group = kn["dma_group"]

    nc = bacc.Bacc("TRN2", target_bir_lowering=False, debug=False,
                   num_devices=N_CORES)

    # One augmented input [4, 147] per core so a single tiny DMA unblocks
    # the whole setup chain:
    #   [:, 0:16]   control-point coords (x strokes 0-7, y strokes 0-7)
    #   [0, 16:19]  [sigma, -1/sqrt2, 1/(RES*sqrt2)]
    #   [:, 19:147] Bernstein basis feat[k, t]
    AUGW = 2 * SPC + 3 + STEPS
    cp_in = nc.dram_tensor("cp_aug", [NK, AUGW], F32, kind="ExternalInput")
    # fp16 output, [stroke, psum-partition j, chunk c, q]; raster row is
    # c*128+j, reassembled on host
    out = nc.dram_tensor("out", [SPC, 128, PCHUNKS * RES], F16,
                         kind="ExternalOutput")

    with tile.TileContext(nc) as tc:
        with tc.tile_pool(name="const", bufs=1) as cpool:
            # Warm the ACT table set (~2.7us load) immediately, overlapping
            # the setup chain: a dep-free Derivative_Erf on a memset tile.
            warm = cpool.tile([1, 1], F32)
            nc.gpsimd.memset(warm[:], 0.0)
            nc.scalar.activation(warm[:], warm[:], AF.Derivative_Erf,
                                 bias=0.0, scale=0.0)

            cp_t = cpool.tile([NK, AUGW], F32)
            nc.sync.dma_start(cp_t[:], cp_in[:])
            feat_t = cp_t[0:NK, 2 * SPC + 3:AUGW]
            # pixel-grid column indices 0..511, on-device; f32 for the
            # activation input path, f16 (exact ints) for the DVE u-build
            g_tile = cpool.tile([128, RES], F32)
            nc.gpsimd.iota(g_tile[:], [[1, RES]], base=0, channel_multiplier=0,
                           allow_small_or_imprecise_dtypes=True)
            g16 = cpool.tile([128, RES], F16)
            nc.gpsimd.iota(g16[:], [[1, RES]], base=0, channel_multiplier=0,
                           allow_small_or_imprecise_dtypes=True)
            ones_t = cpool.tile([1, 128], F32)
            nc.vector.memset(ones_t[:], 1.0)

            # sinv = 1/(sigma*sqrt2);
            # pm = [-sinv (bias scaling), sinv/RES (grid scale)]
            s1 = cpool.tile([1, 1], F32)
            nc.vector.reciprocal(s1[:], cp_t[0:1, 16:17])
            pm = cpool.tile([1, 2], F32)
            nc.vector.tensor_scalar(pm[:], cp_t[0:1, 17:19], s1[:, 0:1], None,
                                    ALU.mult)

            sinv_sb = cpool.tile([128, 2], F32)   # col0=-sinv col1=sinv/RES
            bias_sb = cpool.tile([128, 2 * SPC], F32)  # -sinv * xy_j(t)
            xpx = cpool.tile([128, 2 * SPC], F32)      # RES * xy_j(t)
            with tc.tile_pool(name="spsum", bufs=1, space="PSUM") as spool:
                pbc = spool.tile([128, 2], F32)
                nc.tensor.matmul(pbc[:], lhsT=ones_t[:], rhs=pm[:])

                # raw-cp bias matmul runs parallel to the sinv chain; the
                # -sinv scaling happens in the PSUM->SBUF copy, reading the
                # -sinv scalar straight from PSUM (no staging-copy wait).
                bps = spool.tile([128, 2 * SPC], F32)
                nc.tensor.matmul(bps[:], lhsT=feat_t, rhs=cp_t[:, 0:2 * SPC])
                nc.vector.tensor_scalar(bias_sb[:], bps[:],
                                        pbc[:, 0:1], None,
                                        ALU.mult)
                nc.vector.tensor_scalar(xpx[:], bps[:], float(RES), None,
                                        ALU.mult)
                # sinv staging for the activation-scale AP, after the
                # bias scaling on the in-order DVE queue
                nc.vector.tensor_copy(sinv_sb[:], pbc[:])
            gscaled = cpool.tile([128, RES], F32)
            if kn.get("prescale_grid", False):
                # fold the activation scale into the grid once at setup, so
                # each activation drops one AP operand (scale becomes 1.0)
                nc.vector.tensor_scalar(gscaled[:], g_tile[:],
                                        sinv_sb[:, 1:2], None, ALU.mult)

            if "dmaonly" in probe:
                dsrc = cpool.tile([128, PCHUNKS * RES], F16)
                nc.vector.memset(dsrc[:], 0.25)
                for s in [s for _ in range(repeats) for s in range(SPC)]:
                    nc.sync.dma_start(out[s], dsrc[:])
                rep_iters = []
            else:
                rep_iters = list(range(repeats))

            with tc.tile_pool(name="upool", bufs=kn["upool_bufs"]) as upool, \
                 tc.tile_pool(name="apool", bufs=kn["apool_bufs"]) as apool, \
                 tc.tile_pool(name="opool", bufs=kn["opool_bufs"]) as opool, \
                 tc.tile_pool(name="mmpool", bufs=2, space="PSUM") as mmpool:
                for _ in rep_iters:
                    axy_of = {}     # s -> (ax AP, ay AP)
                    pend_dma = []   # (ot tile, first stroke) awaiting group
                    for s in range(SPC):
                        if big_n:
                            k = s % big_n
                            if k == 0:
                                ut = upool.tile([128, big_n * 2 * RES], F16,
                                                tag="ut")
                                axy = apool.tile([128, big_n * 2 * RES], F16,
                                                 tag="axy")
                                for kk in range(big_n):
                                    ss = s + kk
                                    for xy in range(2):
                                        dst = ut[:, (2 * kk + xy) * RES:
                                                 (2 * kk + xy + 1) * RES]
                                        if "nou" not in probe:
                                            nc.vector.tensor_scalar(
                                                dst, g16[:],
                                                xpx[:, xy * SPC + ss:
                                                    xy * SPC + ss + 1],
                                                sinv_sb[:, 1:2],
                                                ALU.subtract, ALU.mult)
                                    axy_of[ss] = (
                                        axy[:, 2 * kk * RES:
                                            (2 * kk + 1) * RES],
                                        axy[:, (2 * kk + 1) * RES:
                                            (2 * kk + 2) * RES])
                                nc.scalar.activation(axy[:], ut[:],
                                                     AF.Derivative_Erf,
                                                     bias=0.0, scale=1.0)
                            ax, ay = axy_of[s]
                        else:
                            if kn.get("prescale_grid", False):
                                gin, gsc = gscaled[:], 1.0
                            else:
                                gin, gsc = g_tile[:], sinv_sb[:, 1:2]
                            axt = apool.tile([128, RES], F16, tag="ax")
                            nc.scalar.activation(axt[:], gin,
                                                 AF.Derivative_Erf,
                                                 bias=bias_sb[:, s:s + 1],
                                                 scale=gsc)
                            ayt = apool.tile([128, RES], F16, tag="ay")
                            nc.scalar.activation(
                                ayt[:], gin, AF.Derivative_Erf,
                                bias=bias_sb[:, SPC + s:SPC + s + 1],
                                scale=gsc)
                            ax, ay = axt[:], ayt[:]

                        ps = mmpool.tile([128, PCHUNKS * RES], F32, tag="ps")
                        if dma_group == 1 or not pend_dma:
                            ot = opool.tile([128, dma_group * PCHUNKS * RES],
                                            F16, tag="ot")
                        else:
                            ot = pend_dma[-1][0]
                        oview = ot[:, (s % dma_group) * PCHUNKS * RES:
                                   (s % dma_group + 1) * PCHUNKS * RES]
                        ae = kn["drain_split"]
                        early = kn.get("early_act_drain", False)
                        if "nomm" not in probe:
                            for c in range(PCHUNKS):
                                nc.tensor.matmul(
                                    ps[:, c * RES:(c + 1) * RES],
                                    lhsT=ay[:, c * 128:(c + 1) * 128],
                                    rhs=ax)
                                if (early and c == 0 and ae
                                        and "nocopy" not in probe):
                                    # ACT piece only needs chunk 0; emit it
                                    # before the remaining matmuls so the
                                    # scalar queue never stalls on them
                                    nc.scalar.mul(oview[:, 0:ae],
                                                  ps[:, 0:ae], PI_OVER_4)
                        if "nocopy" not in probe:
                            if ae and kn.get("act_drain_tail", False):
                                # ACT takes the tail (needs chunk 3); DVE's
                                # piece [0:2048-ae] covers chunks 0-2 and can
                                # start before the last matmul finishes
                                nw = PCHUNKS * RES - ae
                                nc.vector.tensor_scalar_mul(
                                    oview[:, 0:nw], ps[:, 0:nw], PI_OVER_4)
                                nc.scalar.mul(oview[:, nw:], ps[:, nw:],
                                              PI_OVER_4)
                            elif ae:
                                if not early:
                                    nc.scalar.mul(oview[:, 0:ae],
                                                  ps[:, 0:ae], PI_OVER_4)
                                nc.vector.tensor_scalar_mul(
                                    oview[:, ae:], ps[:, ae:], PI_OVER_4)
                            elif s in kn["act_drain_strokes"]:
                                nc.scalar.mul(oview, ps[:], PI_OVER_4)
                            else:
                                nc.vector.tensor_scalar_mul(oview, ps[:],
                                                            PI_OVER_4)
                        if dma_group == 1:
                            if "nodma" not in probe:
                                nc.sync.dma_start(out[s], ot[:])
                        else:
                            if s % dma_group == 0:
                                pend_dma.append((ot, s))
                            if s % dma_group == dma_group - 1:
                                ot0, s0 = pend_dma.pop()
                                if "nodma" not in probe:
                                    dst = out[s0:s0 + dma_group].rearrange(
                                        "s j w -> j s w")
                                    src = ot0[:].rearrange(
                                        "j (s w) -> j s w", s=dma_group)
                                    # optionally alternate the two physical
                                    # HWDGE rings (SP / ACT) across groups
                                    eng = (nc.scalar
                                           if (kn.get("dma_alt_ring", False)
                                               and (s0 // dma_group) % 2)
                                           else nc.sync)
                                    eng.dma_start(dst, src)

    nc.finalize()
    return nc


_CACHE: dict = {}


def _get_nc() -> bass.Bass:
    if "nc" not in _CACHE:
        _CACHE["nc"] = build_bass()
    return _CACHE["nc"]


def _in_maps(control_points: np.ndarray, sigma) -> list:
    cp = np.asarray(control_points, dtype=np.float32)
    sig = np.float32(np.asarray(sigma).reshape(()))
    isq2 = np.float32(1.0 / SQRT2)
    feat = _bernstein()
    maps = []
    for c in range(N_CORES):
        cpc = cp[BPC * c:BPC * (c + 1)].reshape(SPC, NK, 2)
        cp_aug = np.zeros((NK, 2 * SPC + 3 + STEPS), dtype=np.float32)
        cp_aug[:, :SPC] = cpc[:, :, 0].T
        cp_aug[:, SPC:2 * SPC] = cpc[:, :, 1].T
        cp_aug[0, 2 * SPC] = sig
        cp_aug[0, 2 * SPC + 1] = -isq2
        cp_aug[0, 2 * SPC + 2] = isq2 / np.float32(RES)
        cp_aug[:, 2 * SPC + 3:] = feat
        maps.append({"cp_aug": np.ascontiguousarray(cp_aug)})
    return maps


def run(control_points, sigma, **spmd_kwargs):
    """Run on HW; returns (full_output, BassKernelResults)."""
    nc = _get_nc()
    res = run_bass_kernel_spmd(nc, _in_maps(control_points, sigma),
                               core_ids=list(range(N_CORES)), **spmd_kwargs)
    outs = []
    for r in res.results:
        # [SPC, 128j, PCHUNKS, RES] f16 -> [BPC, S, PCHUNKS, 128j, RES] f32
        a = r["out"].reshape(SPC, 128, PCHUNKS, RES).astype(np.float32)
        a = a.transpose(0, 2, 1, 3).reshape(BPC, S_FULL, RES, RES)
        outs.append(a)
    full = np.concatenate(outs, axis=0)[:, :, None]
    return np.ascontiguousarray(full, dtype=np.float32), res


def kernel(control_points, sigma):
    return run(control_points, sigma)[0]

